# revision 6
# baseline (speedup 1.0000x reference)
"""ConvMultiheadAttention Trainium2 kernel.

Data-parallel over batch: 8 batch elements -> 8 NeuronCores, no collectives.
Per core: conv QKV projections, per-head attention with averaged attention
map output, and the output projection. All matmuls run in fp16 (1 cyc/row on
the PE) with fp32 PSUM accumulation; softmax stats are fp32.

Self-contained: hardcodes shapes; builds + compiles the Bass program once per
process and runs it via bass_utils.run_bass_kernel_spmd on cores 0-7.
"""

import sys

for _p in ("/opt/trn_rl_repo", "/root/.axon_site/_ro/trn_rl_repo"):
    if _p not in sys.path:
        sys.path.append(_p)

import numpy as np

import concourse.bacc as bacc
import concourse.mybir as mybir
import concourse.tile as tile
from concourse import bass_utils

P = 128
D_MODEL = 512
SEQ = 1024
HEADS = 8
HDIM = 64
KW = 3          # conv kernel width (q, k)
DC = D_MODEL // P   # 4 chunks of 128 along d
ST = SEQ // P       # 8 tiles of 128 along s / t
NF = 512            # matmul moving free-dim chunk (one PSUM bank of fp32)
SN = SEQ // NF      # 2

F16 = mybir.dt.float16
F32 = mybir.dt.float32
EXP_BIAS = float(-6.0 * np.log(2.0))  # exp(x/64 - 6ln2): keeps exp in fp16 range
EXP_SCALE = float(1.0 / HDIM)

_CACHE = {}


def _build():
    nc = bacc.Bacc(
        "TRN2", target_bir_lowering=False, debug=False, num_devices=8
    )

    # ---- DRAM I/O (per-core shapes) ----
    xq_d = nc.dram_tensor("xq", [D_MODEL, SEQ + 2], F16, kind="ExternalInput")
    xk_d = nc.dram_tensor("xk", [D_MODEL, SEQ + 2], F16, kind="ExternalInput")
    xv_d = nc.dram_tensor("xv", [D_MODEL, SEQ], F16, kind="ExternalInput")
    wq_d = nc.dram_tensor("wq", [KW, D_MODEL, D_MODEL], F16, kind="ExternalInput")
    wk_d = nc.dram_tensor("wk", [KW, D_MODEL, D_MODEL], F16, kind="ExternalInput")
    wv_d = nc.dram_tensor("wv", [D_MODEL, D_MODEL], F16, kind="ExternalInput")
    wo_d = nc.dram_tensor("wo", [D_MODEL, D_MODEL], F16, kind="ExternalInput")
    bq_d = nc.dram_tensor("bq2", [P, DC], F32, kind="ExternalInput")
    bk_d = nc.dram_tensor("bk2", [P, DC], F32, kind="ExternalInput")
    bfc_d = nc.dram_tensor("bfc2", [P, DC], F32, kind="ExternalInput")
    id_d = nc.dram_tensor("ident", [P, P], F16, kind="ExternalInput")

    outT_d = nc.dram_tensor("outT", [D_MODEL, SEQ], F32, kind="ExternalOutput")
    ave_d = nc.dram_tensor("ave", [SEQ, SEQ], F32, kind="ExternalOutput")

    Exp = mybir.ActivationFunctionType.Exp
    MULT = mybir.AluOpType.mult
    ADD = mybir.AluOpType.add

    with tile.TileContext(nc) as tc:
        with (
            tc.tile_pool(name="static", bufs=1) as static,
            tc.tile_pool(name="epool", bufs=12) as epool,
            tc.tile_pool(name="etpool", bufs=12) as etpool,
            tc.tile_pool(name="spool", bufs=24) as spool,
            tc.tile_pool(name="opool", bufs=3) as opool,
            tc.tile_pool(name="psc", bufs=2, space="PSUM") as psc,
            tc.tile_pool(name="psb", bufs=2, space="PSUM") as psb,
            tc.tile_pool(name="pss", bufs=2, space="PSUM") as pss,
        ):
            # ---- loads ----
            def load(name, shape, dt, src):
                t = static.tile(shape, dt, tag=name, name=name)
                nc.sync.dma_start(t[:], src)
                return t

            xq_s = [load(f"xq{i}", [P, SEQ + 2], F16, xq_d.ap()[i * P:(i + 1) * P, :]) for i in range(DC)]
            wq_s = [[load(f"wq{k}_{i}", [P, D_MODEL], F16, wq_d.ap()[k, i * P:(i + 1) * P, :])
                     for i in range(DC)] for k in range(KW)]
            bq_t = load("bq", [P, DC], F32, bq_d.ap())
            xk_s = [load(f"xk{i}", [P, SEQ + 2], F16, xk_d.ap()[i * P:(i + 1) * P, :]) for i in range(DC)]
            wk_s = [[load(f"wk{k}_{i}", [P, D_MODEL], F16, wk_d.ap()[k, i * P:(i + 1) * P, :])
                     for i in range(DC)] for k in range(KW)]
            bk_t = load("bk", [P, DC], F32, bk_d.ap())
            xv_s = [load(f"xv{i}", [P, SEQ], F16, xv_d.ap()[i * P:(i + 1) * P, :]) for i in range(DC)]
            wv_s = [load(f"wv{i}", [P, D_MODEL], F16, wv_d.ap()[i * P:(i + 1) * P, :]) for i in range(DC)]
            wo_s = [load(f"wo{i}", [P, D_MODEL], F16, wo_d.ap()[i * P:(i + 1) * P, :]) for i in range(DC)]
            bfc_t = load("bfc", [P, DC], F32, bfc_d.ap())
            ident = load("identity", [P, P], F16, id_d.ap())
            ebias = static.tile([P, 1], F32, tag="ebias", name="ebias")
            nc.vector.memset(ebias[:], EXP_BIAS)

            # persistent intermediates
            q_s = [static.tile([P, SEQ], F16, tag=f"q{i}", name=f"q{i}") for i in range(DC)]
            k_s = [static.tile([P, SEQ], F16, tag=f"k{i}", name=f"k{i}") for i in range(DC)]
            vt_s = [static.tile([P, D_MODEL], F16, tag=f"vt{i}", name=f"vt{i}") for i in range(ST)]
            ave_s = [static.tile([P, SEQ], F32, tag=f"ave{i}", name=f"ave{i}") for i in range(ST)]
            a_s = [static.tile([P, D_MODEL], F16, tag=f"a{i}", name=f"a{i}") for i in range(ST)]
            aT_s = [static.tile([P, SEQ], F16, tag=f"aT{i}", name=f"aT{i}") for i in range(DC)]

            # ---- conv projections: Q and K ([d, s], bias per-partition) ----
            for dst, w_s, x_s, b_t in ((q_s, wq_s, xq_s, bq_t), (k_s, wk_s, xk_s, bk_t)):
                for mc in range(DC):
                    for ns in range(SN):
                        ps = psc.tile([P, NF], F32, tag="c", name="c")
                        n = 0
                        for k in range(KW):
                            for ic in range(DC):
                                nc.tensor.matmul(
                                    ps[:],
                                    w_s[k][ic][:, mc * P:(mc + 1) * P],
                                    x_s[ic][:, k + ns * NF:k + ns * NF + NF],
                                    start=(n == 0), stop=(n == KW * DC - 1),
                                )
                                n += 1
                        nc.vector.tensor_scalar_add(
                            dst[mc][:, ns * NF:(ns + 1) * NF], ps[:], b_t[:, mc:mc + 1]
                        )

            # ---- conv V, produced transposed: vt[t, d] (bias folded into fc) ----
            for tt in range(ST):
                ps = psc.tile([P, NF], F32, tag="c", name="c")
                for ic in range(DC):
                    nc.tensor.matmul(
                        ps[:],
                        xv_s[ic][:, tt * P:(tt + 1) * P],
                        wv_s[ic][:],
                        start=(ic == 0), stop=(ic == DC - 1),
                    )
                nc.vector.tensor_copy(vt_s[tt][:], ps[:])

            # ---- attention, one head at a time ----
            for h in range(HEADS):
                hc, off = divmod(h, 2)
                off *= HDIM
                qh = q_s[hc][off:off + HDIM, :]
                kh = k_s[hc][off:off + HDIM, :]

                # scores [s, t] -> exp (fp16) + row sums (fp32)
                e_tiles = []
                recips = []
                for st in range(ST):
                    ps = psb.tile([P, SEQ], F32, tag="b", name="b")
                    for ns in range(SN):
                        nc.tensor.matmul(
                            ps[:, ns * NF:(ns + 1) * NF],
                            qh[:, st * P:(st + 1) * P],
                            kh[:, ns * NF:(ns + 1) * NF],
                            start=True, stop=True,
                        )
                    et = epool.tile([P, SEQ], F16, tag="e", name="e")
                    sums = spool.tile([P, 1], F32, tag="sum", name="sum")
                    nc.scalar.activation(
                        et[:], ps[:], Exp, bias=ebias[:], scale=EXP_SCALE,
                        accum_out=sums[:],
                    )
                    rec = spool.tile([P, 1], F32, tag="rec", name="rec")
                    nc.vector.reciprocal(rec[:], sums[:])
                    e_tiles.append(et)
                    recips.append(rec)

                # ave_att accumulation: ave[s,t] += exp * recip  (DVE fused)
                for st in range(ST):
                    if h == 0:
                        nc.vector.tensor_scalar_mul(
                            ave_s[st][:], e_tiles[st][:], recips[st][:]
                        )
                    else:
                        nc.vector.scalar_tensor_tensor(
                            ave_s[st][:], e_tiles[st][:], recips[st][:],
                            ave_s[st][:], MULT, ADD,
                        )

                # scores transposed [t, s] -> exp (fp16)
                eT_tiles = []
                for tt in range(ST):
                    ps = psb.tile([P, SEQ], F32, tag="b", name="b")
                    for ns in range(SN):
                        nc.tensor.matmul(
                            ps[:, ns * NF:(ns + 1) * NF],
                            kh[:, tt * P:(tt + 1) * P],
                            qh[:, ns * NF:(ns + 1) * NF],
                            start=True, stop=True,
                        )
                    etT = etpool.tile([P, SEQ], F16, tag="eT", name="eT")
                    nc.scalar.activation(
                        etT[:], ps[:], Exp, bias=ebias[:], scale=EXP_SCALE
                    )
                    eT_tiles.append(etT)

                # attn @ V: out[s, d_h] = sum_t expT[t,s] * vt[t,d_h], then 1/sum
                for st in range(ST):
                    pa = pss.tile([P, HDIM], F32, tag="s", name="s")
                    for tt in range(ST):
                        nc.tensor.matmul(
                            pa[:],
                            eT_tiles[tt][:, st * P:(st + 1) * P],
                            vt_s[tt][:, h * HDIM:(h + 1) * HDIM],
                            start=(tt == 0), stop=(tt == ST - 1),
                        )
                    nc.vector.tensor_scalar_mul(
                        a_s[st][:, h * HDIM:(h + 1) * HDIM], pa[:], recips[st][:]
                    )

            # ---- finalize ave (mean over heads) + store ----
            for st in range(ST):
                nc.vector.tensor_scalar_mul(ave_s[st][:], ave_s[st][:], 1.0 / HEADS)
                nc.sync.dma_start(ave_d.ap()[st * P:(st + 1) * P, :], ave_s[st][:])

            # ---- transpose a [s, d] -> aT [d, s] via PE ----
            for dc in range(DC):
                for st in range(ST):
                    pt = pss.tile([P, P], F16, tag="s", name="s")
                    nc.tensor.transpose(
                        pt[:], a_s[st][:, dc * P:(dc + 1) * P], ident[:]
                    )
                    nc.vector.tensor_copy(aT_s[dc][:, st * P:(st + 1) * P], pt[:])

            # ---- fc_out: outT[o, s] = sum_d wo[d, o] * aT[d, s] + bfc ----
            for oc in range(DC):
                for ns in range(SN):
                    pf = psc.tile([P, NF], F32, tag="c", name="c")
                    for dc in range(DC):
                        nc.tensor.matmul(
                            pf[:],
                            wo_s[dc][:, oc * P:(oc + 1) * P],
                            aT_s[dc][:, ns * NF:(ns + 1) * NF],
                            start=(dc == 0), stop=(dc == DC - 1),
                        )
                    ot = opool.tile([P, NF], F32, tag="o", name="o")
                    nc.vector.tensor_scalar_add(ot[:], pf[:], bfc_t[:, oc:oc + 1])
                    nc.sync.dma_start(
                        outT_d.ap()[oc * P:(oc + 1) * P, ns * NF:(ns + 1) * NF], ot[:]
                    )

    nc.compile()
    return nc


def _prep_in_maps(query, key_t, value, Wq, bq, Wk, bk, Wv, bv, Wo, bo):
    f16 = np.float16
    f32 = np.float32
    B = query.shape[0]

    def padT(x):  # [S, D] -> [D, S+2] fp16, zero edges
        p = np.zeros((D_MODEL, SEQ + 2), f16)
        p[:, 1:SEQ + 1] = x.T.astype(f16)
        return p

    # weights: transposed to [in, out] per tap
    wq = np.ascontiguousarray(np.transpose(Wq, (2, 1, 0))).astype(f16)  # [K, i, o]
    wk = np.ascontiguousarray(np.transpose(Wk, (2, 1, 0))).astype(f16)
    wv = np.ascontiguousarray(Wv[:, :, 0].T).astype(f16)                # [i, o]
    wo = np.ascontiguousarray(Wo.T).astype(f16)                         # [d, o]
    b_fc = (bo.astype(f32) + Wo.astype(f32) @ bv.astype(f32))

    def cols(b):  # [512] -> [128, 4] col-per-chunk, fp32
        return np.ascontiguousarray(b.astype(f32).reshape(DC, P).T)

    shared = {
        "wq": wq, "wk": wk, "wv": wv, "wo": wo,
        "bq2": cols(bq), "bk2": cols(bk), "bfc2": cols(b_fc),
        "ident": np.eye(P, dtype=f16),
    }
    in_maps = []
    for b in range(B):
        m = dict(shared)
        m["xq"] = padT(np.asarray(query[b], f32))
        m["xk"] = padT(np.asarray(key_t[b], f32))
        m["xv"] = np.ascontiguousarray(np.asarray(value[b], f32).T).astype(f16)
        in_maps.append(m)
    return in_maps


def kernel(query, key_t, value, Wq, bq, Wk, bk, Wv, bv, Wo, bo):
    if "nc" not in _CACHE:
        _CACHE["nc"] = _build()
    nc = _CACHE["nc"]

    args = [np.asarray(a) for a in (query, key_t, value, Wq, bq, Wk, bk, Wv, bv, Wo, bo)]
    in_maps = _prep_in_maps(*args)
    res = bass_utils.run_bass_kernel_spmd(nc, in_maps, core_ids=list(range(8)))

    B = args[0].shape[0]
    out = np.stack([np.ascontiguousarray(res.results[b]["outT"].T) for b in range(B)])
    ave = np.stack([res.results[b]["ave"] for b in range(B)])
    return out.astype(np.float32), ave.astype(np.float32)
